# revision 17
# baseline (speedup 1.0000x reference)
"""Trainium2 Bass kernel for nn_CombinedLoss (regression MSE + masked binary focal loss).

Data-parallel over 8 NeuronCores: each core reduces its batch shard to
per-class partial sums; the final (tiny) weighted combination happens on host
in float64.

Math (per element of the 13 presence classes, t in {-1, 0, 1}):
    z = x*(1 - 2t) + 40*min(t, 0)       (= x, -x, 3x - 40)
    focal(x, t) = softplus(z) * sigmoid(z)^2
For t == -1 the bias -40 drives z <= -22, so exp(z) < 1.3e-10 and
ln(1 + exp(z)) rounds to exactly 0 in fp32: masked entries contribute
nothing (f == 0 exactly). Only TWO per-class sums are then needed:
    S0_c = sum f          Sh_c = sum f*t   (only t==1 survives)
Host side:  F1 = Sh,  F0 = S0 - Sh
    focal_total = sum_c (1-w_c)*F0_c + w_c*F1_c

softplus/sigmoid^2 use only the `natural_log_exp_and_others` ACT table set:
    e  = exp(z)            [ACT Exp, fp32]
    sp = ln(e + 1)         [ACT Ln, bias=+1]   == softplus(z), bf16
    s2 = exp(2*(z - sp))   [ACT Exp, scale=2]  == sigmoid(z)^2, bf16
so the ACT engine never switches table sets (one ACT_TABLE_LOAD total).

Measured HW rates drove the op selection: every 2-source DVE tensor_tensor
costs ~0.61 ns/elem regardless of dtype (SBUF-port-bound; bf16 2x does not
materialize), tensor_scalar is ~half that for bf16 sources, and
scalar_tensor_tensor is ~2x WORSE (two ALU passes) — so the kernel uses
only tensor_tensor/tensor_scalar. Per group of 2 tiles (3328 class elems,
~5.7-6.3us DMA):
    DVE    a,u (TS) + z,v,f,h,d (TT)    ~6.7us
    ACT    3 ops                        ~5.1us
    GPSIMD zm = x*a, q = d*d            ~6.3us
    PE     2 class streams + reg (9 mm) ~5.8us
"""

import sys

if "/opt/trn_rl_repo" not in sys.path:
    sys.path.insert(0, "/opt/trn_rl_repo")

import numpy as np

NCORES = 8
B = 2_097_152
BS = B // NCORES          # 262144 rows per core
P = 128                   # SBUF partitions
RPP = BS // P             # 2048 rows per partition
T = 64                    # rows per tile
NT = RPP // T             # 32 tiles
FD_FULL = T * 16          # 1024 fp32 per partition per tile (4KB DMA run)
FD_C = T * 13             # 832 class elements per partition per tile
FD_R = T * 3              # 192 regression elements per partition per tile
GRP = 2                   # tiles per group (g build param must match)
NPART = 2 * FD_C + GRP * FD_R   # 2048 partial-sum cells per core


def build(reps: int = 1, g: int = 2, bufs_io: int = 4, bufs32: int = 3,
          bufs16: int = 3, dma_split: int = 1):
    import concourse.bacc as bacc
    import concourse.mybir as mybir
    import concourse.tile as tile
    import bass_rust as _bass_rust
    from concourse.hw_specs import get_activation_tables

    dt = mybir.dt
    AF = mybir.ActivationFunctionType
    OP = mybir.AluOpType

    class _Bacc(bacc.Bacc):
        """Pin every activation to the natural_log_exp_and_others table set.

        The default chooser scans act_func_sets in order and picks the first
        set containing each function, so Exp lands in exp_and_others and Ln
        in natural_log — alternating ACT_TABLE_LOADs (~1.3us each) every
        group. All functions this kernel uses (Exp, Ln, Copy) live together
        in natural_log_exp_and_others; blanking the other sets (positions
        preserved, since act_func_set_id is the list index) yields exactly
        one table load for the whole kernel.
        """

        def insert_act_table_loads(self):
            has_activation = any(
                isinstance(i, mybir.InstActivation)
                for b in self.main_func.blocks
                for i in b.instructions
            )
            if not has_activation:
                return
            keep = "natural_log_exp_and_others"
            tables = [
                (name, funcs if name == keep else set())
                for name, funcs in get_activation_tables(self.m.arch).items()
            ]
            _bass_rust.insert_act_table_loads(self, tables)

    G_ = g
    NGROUP_ = NT // G_
    FDGF = G_ * FD_FULL
    FDGC = G_ * FD_C
    FDGR = G_ * FD_R

    nc = _Bacc("TRN2", target_bir_lowering=False, debug=False,
               num_devices=NCORES)
    x_d = nc.dram_tensor("output", [BS, 16], dt.float32, kind="ExternalInput")
    t_d = nc.dram_tensor("target", [BS, 16], dt.float32, kind="ExternalInput")
    po_d = nc.dram_tensor("partials", [1, NPART], dt.float32,
                          kind="ExternalOutput")

    # [128, 32768] per-partition contiguous row blocks
    xv = x_d.ap().rearrange("(p r) c -> p (r c)", p=P)
    tv = t_d.ap().rearrange("(p r) c -> p (r c)", p=P)

    with tile.TileContext(nc) as tc:
        with (
            tc.tile_pool(name="io", bufs=bufs_io) as io_pool,
            tc.tile_pool(name="f32", bufs=bufs32) as f32_pool,
            tc.tile_pool(name="b16", bufs=bufs16) as b16_pool,
            tc.tile_pool(name="cst", bufs=1) as cst_pool,
            tc.tile_pool(name="acc", bufs=1, space="PSUM") as psum_pool,
        ):
            ones = cst_pool.tile([P, 1], dt.bfloat16, tag="ones")
            nc.vector.memset(ones[:], 1.0)

            p0 = psum_pool.tile([1, FD_C], dt.float32, tag="p0")
            p1 = psum_pool.tile([1, FD_C], dt.float32, tag="p1")
            pq = psum_pool.tile([1, FDGR], dt.float32, tag="pq")

            for rep in range(reps):
                for gi in range(NGROUP_):
                    xg = io_pool.tile([P, FDGF], dt.float32, tag="xg")
                    tg = io_pool.tile([P, FDGF], dt.float32, tag="tg")
                    step = FDGF // dma_split
                    for i in range(dma_split):
                        sl_s = slice(i * step, (i + 1) * step)
                        sl_d = slice(gi * FDGF + i * step,
                                     gi * FDGF + (i + 1) * step)
                        nc.sync.dma_start(xg[:, sl_s], xv[:, sl_d])
                        nc.sync.dma_start(tg[:, sl_s], tv[:, sl_d])

                    x3 = xg[:].rearrange("p (r c) -> p r c", c=16)
                    t3 = tg[:].rearrange("p (r c) -> p r c", c=16)
                    xc, tc_v = x3[:, :, 3:16], t3[:, :, 3:16]
                    xr, tr_v = x3[:, :, 0:3], t3[:, :, 0:3]

                    # a = 1 - 2t  (exact {3, 1, -1} in bf16)
                    ag = b16_pool.tile([P, FDGC], dt.bfloat16, tag="a")
                    nc.vector.tensor_scalar(
                        ag[:], tc_v, -2.0, 1.0, OP.mult, OP.add)
                    # r = relu(-40t): exact {40, 0, 0} — the mask bias,
                    # built on ACT (same table set) to spare a DVE op.
                    # GPSIMD is left idle on purpose: it shares SBUF ports
                    # with DVE and degrades concurrent DVE ops ~4x.
                    rg = b16_pool.tile([P, FDGC], dt.bfloat16, tag="r")
                    nc.scalar.activation(rg[:], tc_v, AF.Relu, scale=-40.0)
                    # zm = x*a
                    zg = b16_pool.tile([P, FDGC], dt.bfloat16, tag="z")
                    nc.vector.tensor_tensor(zg[:], xc, ag[:], OP.mult)
                    # z = zm - r  (masked z)
                    z2g = b16_pool.tile([P, FDGC], dt.bfloat16, tag="z2")
                    nc.vector.tensor_tensor(z2g[:], zg[:], rg[:], OP.subtract)

                    # e = exp(z) in fp32 so ln(1+e) underflows to exactly 0
                    # for masked entries (e < 1.3e-10)
                    eg = f32_pool.tile([P, FDGC], dt.float32, tag="e")
                    nc.scalar.activation(eg[:], z2g[:], AF.Exp)
                    # sp = ln(e + 1) = softplus(z), bf16
                    spg = b16_pool.tile([P, FDGC], dt.bfloat16, tag="sp")
                    nc.scalar.activation(spg[:], eg[:], AF.Ln, bias=1.0)
                    # v = z - sp
                    vg = b16_pool.tile([P, FDGC], dt.bfloat16, tag="v")
                    nc.vector.tensor_tensor(vg[:], z2g[:], spg[:], OP.subtract)
                    # s2 = exp(2v) = sigmoid(z)^2
                    s2g = b16_pool.tile([P, FDGC], dt.bfloat16, tag="s2")
                    nc.scalar.activation(s2g[:], vg[:], AF.Exp, scale=2.0)

                    fg = b16_pool.tile([P, FDGC], dt.bfloat16, tag="f")
                    nc.vector.tensor_tensor(fg[:], spg[:], s2g[:], OP.mult)
                    # h = f*a (packed bf16); host: F1 = (S0-Sa)/2
                    hg = b16_pool.tile([P, FDGC], dt.bfloat16, tag="h")
                    nc.vector.tensor_tensor(hg[:], fg[:], ag[:], OP.mult)

                    # regression: q = (x - t)^2 in bf16
                    dg = b16_pool.tile([P, FDGR], dt.bfloat16, tag="d")
                    nc.vector.tensor_tensor(dg[:], xr, tr_v, OP.subtract)
                    qg = b16_pool.tile([P, FDGR], dt.bfloat16, tag="q")
                    nc.vector.tensor_tensor(qg[:], dg[:], dg[:], OP.mult)

                    for i in range(G_):
                        j = gi * G_ + i
                        st = j == 0
                        fin = j == NT - 1
                        off = i * FD_C
                        for (acc, src) in ((p0, fg), (p1, hg)):
                            nc.tensor.matmul(acc[:, 0:512], ones[:],
                                             src[:, off:off + 512],
                                             start=st, stop=fin)
                            nc.tensor.matmul(acc[:, 512:FD_C], ones[:],
                                             src[:, off + 512:off + FD_C],
                                             start=st, stop=fin)
                    # single q matmul per group into a [1, G*FD_R] psum
                    nc.tensor.matmul(pq[:], ones[:], qg[:],
                                     start=(gi == 0),
                                     stop=(gi == NGROUP_ - 1))

            outt = cst_pool.tile([1, NPART], dt.float32, tag="out")
            nc.scalar.copy(outt[:, 0:FD_C], p0[:])
            nc.scalar.copy(outt[:, FD_C:2 * FD_C], p1[:])
            nc.scalar.copy(outt[:, 2 * FD_C:NPART], pq[:])
            nc.sync.dma_start(po_d.ap(), outt[:])

    nc.compile()
    return nc


# ---------------------------------------------------------------------------
# Cached PJRT executor (jit once per process; later calls are cheap).
# Mirrors concourse.bass2jax.run_bass_via_pjrt for the 8-core SPMD case.
# ---------------------------------------------------------------------------

_EXEC = None


def _get_executor():
    global _EXEC
    if _EXEC is not None:
        return _EXEC

    import jax
    import concourse.mybir as mybir
    from concourse import bass2jax
    from jax.sharding import Mesh, PartitionSpec
    from jax.experimental.shard_map import shard_map

    nc = build(1)
    bass2jax.install_neuronx_cc_hook()

    partition_name = (nc.partition_id_tensor.name
                      if nc.partition_id_tensor else None)
    in_names, out_names, out_avals = [], [], []
    for alloc in nc.m.functions[0].allocations:
        if not isinstance(alloc, mybir.MemoryLocationSet):
            continue
        name = alloc.memorylocations[0].name
        if alloc.kind == "ExternalInput":
            if name != partition_name:
                in_names.append(name)
        elif alloc.kind == "ExternalOutput":
            out_names.append(name)
            out_avals.append(jax.core.ShapedArray(
                tuple(alloc.tensor_shape), mybir.dt.np(alloc.dtype)))

    n_params = len(in_names)
    n_outs = len(out_avals)
    all_in_names = list(in_names) + list(out_names)
    if partition_name is not None:
        all_in_names.append(partition_name)

    def _body(*args):
        operands = list(args)
        if partition_name is not None:
            operands.append(bass2jax.partition_id_tensor())
        return tuple(bass2jax._bass_exec_p.bind(
            *operands,
            out_avals=tuple(out_avals),
            in_names=tuple(all_in_names),
            out_names=tuple(out_names),
            lowering_input_output_aliases=(),
            sim_require_finite=True,
            sim_require_nnan=True,
            nc=nc,
        ))

    devices = jax.devices()[:NCORES]
    mesh = Mesh(np.asarray(devices), ("core",))
    in_specs = (PartitionSpec("core"),) * (n_params + n_outs)
    out_specs = (PartitionSpec("core"),) * n_outs
    donate = tuple(range(n_params, n_params + n_outs))
    sharded = jax.jit(
        shard_map(_body, mesh=mesh, in_specs=in_specs, out_specs=out_specs,
                  check_rep=False),
        donate_argnums=donate, keep_unused=True)

    _EXEC = (sharded, in_names, out_names, out_avals)
    return _EXEC


def run_device_partials(output: np.ndarray, target: np.ndarray) -> np.ndarray:
    """Run the SPMD kernel; returns per-core partials [NCORES, NPART] fp32."""
    sharded, in_names, out_names, out_avals = _get_executor()
    feeds = {"output": np.ascontiguousarray(output, dtype=np.float32),
             "target": np.ascontiguousarray(target, dtype=np.float32)}
    ins = [feeds[n] for n in in_names]
    zeros = [np.zeros((NCORES * a.shape[0],) + a.shape[1:], a.dtype)
             for a in out_avals]
    outs = sharded(*ins, *zeros)
    idx = out_names.index("partials")
    return np.asarray(outs[idx]).reshape(NCORES, NPART)


def combine_partials(partials: np.ndarray,
                     binary_class_weights: np.ndarray) -> np.float32:
    """Host-side fp64 combination of per-core partial sums into the loss."""
    p = partials.astype(np.float64).sum(axis=0)
    S0 = p[0:FD_C].reshape(T, 13).sum(axis=0)
    Sa = p[FD_C:2 * FD_C].reshape(T, 13).sum(axis=0)
    Q = p[2 * FD_C:NPART].reshape(GRP * T, 3).sum(axis=0)
    w = np.asarray(binary_class_weights, dtype=np.float64)
    F1 = (S0 - Sa) / 2.0
    F0 = (S0 + Sa) / 2.0
    focal = np.sum((1.0 - w) * F0 + w * F1)
    mse = Q / float(B)
    loss = 10.0 * mse[0] + mse[1] + mse[2] + focal
    return np.float32(loss)


def kernel(output: np.ndarray, target: np.ndarray,
           binary_class_weights: np.ndarray) -> np.ndarray:
    partials = run_device_partials(output, target)
    return np.asarray(combine_partials(partials, binary_class_weights))
